# revision 3
# baseline (speedup 1.0000x reference)
import numpy as np
import ml_dtypes
import jax
import jax.numpy as jnp
from jax import lax

# Binarized CNN forward (nn_BCNN): conv1(VALID, sign(w1)) -> pool -> BN, then
# 3 blocks of sign(y) conv sign(w) SAME -> pool -> BN.
# Data-parallel over the batch dim: 64 images -> 8 shards of 8, one per NeuronCore.
#
# Numerics: sign(w) and sign(y) are exactly representable in bf16, and conv
# accumulation is forced to fp32 (preferred_element_type), so the binarized
# convs (2-4) are bit-exact integer sums. conv1 uses an exact 3-way bf16
# split of x (x == hi+mid+lo exactly for fp32 inputs). The split is computed
# on the host and fed as three separate inputs (with separate weight buffers)
# so the compiler cannot algebraically re-merge it into one bf16 conv.

BN_EPS = np.float32(1e-3)
_BF = jnp.bfloat16
_F32 = jnp.float32


def _sign(x):
    return jnp.where(x >= 0, jnp.ones_like(x), -jnp.ones_like(x))


def _conv(x, w, padding):
    return lax.conv_general_dilated(
        x, w, window_strides=(1, 1), padding=padding,
        dimension_numbers=('NHWC', 'HWIO', 'NHWC'),
        preferred_element_type=_F32)


def _maxpool2(x):
    return lax.reduce_window(x, -jnp.inf, lax.max, (1, 2, 2, 1), (1, 2, 2, 1), 'VALID')


def _bn(x, mean, var, beta):
    return (x - mean) * lax.rsqrt(var + BN_EPS) + beta


def _forward(hi, mid, lo, s1a, s1b, s1c,
             m1, v1, b1, w2, m2, v2, b2, w3, m3, v3, b3, w4, m4, v4, b4):
    y = (_conv(hi, s1a, 'VALID') + _conv(mid, s1b, 'VALID')
         + _conv(lo, s1c, 'VALID'))
    y = _bn(_maxpool2(y), m1, v1, b1)
    for w, m, v, b in ((w2, m2, v2, b2), (w3, m3, v3, b3), (w4, m4, v4, b4)):
        y = _conv(_sign(y).astype(_BF), _sign(w).astype(_BF), 'SAME')
        y = _bn(_maxpool2(y), m, v, b)
    return y


_N_CORES = 8
_pforward = jax.pmap(_forward, in_axes=(0, 0, 0) + (None,) * 18)


def kernel(**inputs):
    x = np.asarray(inputs['x'], dtype=np.float32)
    b = x.shape[0]

    bf = ml_dtypes.bfloat16
    hi = x.astype(bf)
    r1 = x - hi.astype(np.float32)
    mid = r1.astype(bf)
    lo = (r1 - mid.astype(np.float32)).astype(bf)

    def shard(t):
        return t.reshape(_N_CORES, b // _N_CORES, *t.shape[1:])

    s1 = np.where(np.asarray(inputs['w1'], np.float32) >= 0, 1, -1).astype(bf)
    ws = [np.asarray(inputs[k], dtype=np.float32) for k in
          ('m1', 'v1', 'b1', 'w2', 'm2', 'v2', 'b2',
           'w3', 'm3', 'v3', 'b3', 'w4', 'm4', 'v4', 'b4')]
    out = _pforward(shard(hi), shard(mid), shard(lo),
                    s1, s1.copy(), s1.copy(), *ws)
    out = np.asarray(out, dtype=np.float32)
    return out.reshape(b, *out.shape[2:])
